# revision 6
# baseline (speedup 1.0000x reference)
"""BQQ linear inference kernel for 8 Trainium2 NeuronCores.

Math: after activation quantization, the whole BQQ op is linear in the
quantized input, so all four correction terms fold into one weight matrix:

    out[b, (j,m)] = X[b, (k,n)] @ W'[(k,n), (j,m)] + bias

where X = clip(round(x / act_scale), -127, 127) * act_scale and W' is a pure
function of the weights (Y_sign/Z_sign/scales/A), folded on the host.  The
device kernel per core is a pure streaming GEMM, tensor-parallel over the j
(output block) dim: 4 of 32 j-blocks per core.

Schedule notes (from trace analysis): the kernel is tensor-engine bound
(128 matmuls x ~216 ns).  HWDGE descriptor generation runs at ~1 descriptor
per 19 ns per ring (one descriptor per partition-row per trigger), so the
input stream uses FEW large triggers with long per-partition runs: x^T (int8)
and W (bf16, raw bytes) are interleaved per-k in one int8-typed DRAM tensor
per ring, chunks of 2-4 k-slabs, rings alternating k-pairs so slabs land in
k-order.  k0/k1 ship x as bf16 so the first matmuls need no cast; later x
slabs are upcast int8->bf16 on the DVE off the critical path (ints <= 127
are exact in bf16).  W regions are read through bf16 bitcast views.  A short
warmup matmul chain holds the PE clock (p-state) up through the DMA wait
without delaying the first real matmul.
"""

import numpy as np
import ml_dtypes

import concourse.bass as bass
import concourse.bacc as bacc
import concourse.mybir as mybir
from concourse.tile import TileContext
from concourse.tile_rust import add_dep_helper
from concourse.bass_utils import run_bass_kernel_spmd

F32 = mybir.dt.float32
BF16 = mybir.dt.bfloat16
I8 = mybir.dt.int8

P_, J, K, M, L, N = 2, 32, 32, 128, 16, 128
B = 512                  # tokens
NCORES = 8
JLOC = J // NCORES       # 4 j-blocks per core
CPJ = JLOC * M           # 512 output cols per core
QMAX = 127.0
WARMUP = 52
KSPLIT = 24              # k < KSPLIT: banks interleaved; then bank-by-bank

# rings alternate 4-k chunks (each chunk = one trigger = 128 descriptors,
# ~2.4 us of descriptor generation); ring A carries k0-3 so the critical
# start avoids ring B's slower first-packet latency.
CHUNKS_A = [[0, 1, 2, 3], [8, 9, 10, 11], [16, 17, 18, 19],
            [24, 25, 26, 27]]
CHUNKS_B = [[4, 5, 6, 7], [12, 13, 14, 15], [20, 21, 22, 23],
            [28, 29, 30, 31]]
XBF = (0, 1)             # k-slabs whose x part ships as bf16 (2048 B slabs)


def _slab_bytes(k):
    return 2048 if k in XBF else 1536   # [x | w] bytes per partition


def _layout():
    """k -> (ring, byte_base) plus per-ring total bytes."""
    where = {}
    tot = [0, 0]
    for r, chunks in enumerate((CHUNKS_A, CHUNKS_B)):
        for ch in chunks:
            for k in ch:
                where[k] = (r, tot[r])
                tot[r] += _slab_bytes(k)
    return where, tot


WHERE, RING_BYTES = _layout()
BIAS_BASE = RING_BYTES[1]
RING_BYTES = [RING_BYTES[0], RING_BYTES[1] + 2 * CPJ]   # bias on ring B

_CACHE = {}


def _build_bass():
    nc = bacc.Bacc()
    a_d = nc.declare_dram_parameter("ring_a", [128, RING_BYTES[0]], I8,
                                    isOutput=False)
    b_d = nc.declare_dram_parameter("ring_b", [128, RING_BYTES[1]], I8,
                                    isOutput=False)
    out_d = nc.declare_dram_parameter("out", [B, CPJ], BF16, isOutput=True)

    with TileContext(nc) as tc:
        with tc.tile_pool(name="big", bufs=1) as big, \
             tc.tile_pool(name="sm", bufs=1) as sm, \
             tc.tile_pool(name="ot", bufs=4) as ot, \
             tc.tile_pool(name="psum", bufs=1, space="PSUM") as pp:
            pa = big.tile([128, RING_BYTES[0]], I8)    # ring A slabs
            pb = big.tile([128, RING_BYTES[1]], I8)    # ring B slabs + bias
            xbt = big.tile([128, K * B], BF16)         # upcast x^T (k >= 2)
            wz = sm.tile([128, 192], BF16)             # zeros for warmup
            wzms = nc.gpsimd.memset(wz[:], 0.0)

            psums = [pp.tile([128, CPJ], F32, name=f"psum{i}", tag=f"psum{i}")
                     for i in range(4)]
            wps = pp.tile([128, 64], F32, name="wps", tag="wps")

            def ring(k):
                r, base = WHERE[k]
                return (pa if r == 0 else pb), base

            # Phase A: a few big escalating triggers per ring, k-ascending.
            for tiles, dram, chunks in ((pa, a_d, CHUNKS_A), (pb, b_d,
                                                             CHUNKS_B)):
                eng = nc.sync if dram is a_d else nc.scalar
                for ch in chunks:
                    lo = WHERE[ch[0]][1]
                    hi = WHERE[ch[-1]][1] + _slab_bytes(ch[-1])
                    eng.dma_start(out=tiles[:, lo:hi], in_=dram[:, lo:hi])
            # bias (needed only at the tail) rides ring B last
            nc.scalar.dma_start(out=pb[:, BIAS_BASE:],
                                in_=b_d[:, BIAS_BASE:])
            bias_bc = pb[:, BIAS_BASE:].bitcast(BF16)

            # warmup matmuls paced off an early memset keep the PE p-state
            # ramped through the DMA wait
            for _ in range(WARMUP):
                mm = nc.tensor.matmul(
                    wps[:], lhsT=wz[:, 0:128],
                    rhs=wz[:, 128:192], start=True, stop=True)
                add_dep_helper(mm.ins, wzms.ins,
                               reason="pace PE warmup after memset")

            # x upcasts for k >= 2, in k order (k0/k1 are bf16 already)
            for k in range(2, K):
                t, base = ring(k)
                nc.vector.tensor_copy(
                    out=xbt[:, k * B:(k + 1) * B],
                    in_=t[:, base:base + 512])

            def lhsT(k, bb):
                if k in XBF:
                    t, base = ring(k)
                    return t[:, base + bb * 256:base + (bb + 1) * 256
                             ].bitcast(BF16)
                return xbt[:, k * B + bb * 128:k * B + (bb + 1) * 128]

            def rhs(k):
                t, base = ring(k)
                wb = base + (1024 if k in XBF else 512)
                return t[:, wb:wb + 1024].bitcast(BF16)

            # Phase B: the GEMM k-loop.  The last K - KSPLIT steps run
            # bank-by-bank so bank bb's epilogue (bias add + bf16 cast)
            # starts while bank bb+1 is still accumulating.
            def mm_step(k, bb, stop):
                nc.tensor.matmul(psums[bb][:], lhsT=lhsT(k, bb), rhs=rhs(k),
                                 start=(k == 0), stop=stop)

            def epilogue(bb):
                o = ot.tile([128, CPJ], BF16)
                nc.vector.tensor_add(o[:], psums[bb][:], bias_bc)
                rows = slice(bb * 128, (bb + 1) * 128)
                nc.sync.dma_start(out=out_d[rows, 0:256], in_=o[:, 0:256])
                nc.scalar.dma_start(out=out_d[rows, 256:512],
                                    in_=o[:, 256:512])

            for k in range(KSPLIT):
                for bb in range(4):
                    mm_step(k, bb, stop=False)
            for bb in range(4):
                for k in range(KSPLIT, K):
                    mm_step(k, bb, stop=(k == K - 1))
                epilogue(bb)
    return nc


def _fold_weights(Y_sign, Z_sign, Y_scale, Z_scale, A):
    """W[j,k,n,m]: everything linear in X folded into one matrix (fp32)."""
    ysc = Y_scale[..., 0, 0].astype(np.float32)      # (p,j,k)
    zsc = Z_scale[..., 0, 0].astype(np.float32)
    a0, a1, a2, a3 = (A[..., i].astype(np.float32) for i in range(4))
    Zs = Z_sign.astype(np.float32)
    Ys = Y_sign.astype(np.float32)
    # out1: sum_{p,l} a0*ysc*zsc * Z[l,n] * Y[m,l]  -> (j,k,n,m)
    t1 = np.einsum('pjkln,pjkml->pjknm', Zs, Ys, optimize=True)
    W = np.einsum('pjk,pjknm->jknm', a0 * ysc * zsc, t1, optimize=True)
    # out2: B_coef[j,k,m] broadcast over n
    Ysum = Ys.sum(-1) * ysc[..., None]               # (p,j,k,m)
    W += np.einsum('pjk,pjkm->jkm', a1, Ysum)[:, :, None, :]
    # out3: sum_p a2*zsc*Zsum[n] broadcast over m
    Zsum = Zs.sum(-2) * zsc[..., None]               # (p,j,k,n)
    W += np.einsum('pjk,pjkn->jkn', a2, Zsum)[:, :, :, None]
    # out4: D_coef[j,k] broadcast over n,m
    W += a3.sum(0)[:, :, None, None]
    return W


def _prepare(inputs):
    x = np.asarray(inputs["input"], dtype=np.float32)
    W = _fold_weights(np.asarray(inputs["Y_sign"], np.float32),
                      np.asarray(inputs["Z_sign"], np.float32),
                      np.asarray(inputs["Y_scale"], np.float32),
                      np.asarray(inputs["Z_scale"], np.float32),
                      np.asarray(inputs["A"], np.float32))
    bias = np.asarray(inputs["bias"], np.float32)

    # activation quantization on host (exact global max/min, RNE round)
    act_scale = max((float(x.max()) - float(x.min())) / (2.0 * QMAX), 1e-8)
    xq = np.clip(np.round(x / act_scale), -QMAX, QMAX)
    W = W * act_scale    # fold act_scale into the weights

    xtT = xq.reshape(B, K, N).transpose(2, 1, 0)     # [n, k, b] fp32
    x8 = xtT.astype(np.int8).view(np.uint8)          # int8 bytes
    xh = np.ascontiguousarray(xtT.astype(ml_dtypes.bfloat16)).view(np.uint8)

    in_maps = []
    for cid in range(NCORES):
        Wc = W[cid * JLOC:(cid + 1) * JLOC]          # [jl,k,n,m]
        wgt = np.ascontiguousarray(
            Wc.transpose(2, 1, 0, 3).reshape(N, K, CPJ).astype(
                ml_dtypes.bfloat16)).view(np.uint8)  # [n, k, 1024 bytes]
        rings = [np.empty((N, RING_BYTES[r]), np.uint8) for r in range(2)]
        for k in range(K):
            r, base = WHERE[k]
            if k in XBF:
                rings[r][:, base:base + 1024] = xh[:, k]
                rings[r][:, base + 1024:base + 2048] = wgt[:, k]
            else:
                rings[r][:, base:base + 512] = x8[:, k]
                rings[r][:, base + 512:base + 1536] = wgt[:, k]
        rings[1][:, BIAS_BASE:] = np.ascontiguousarray(np.broadcast_to(
            bias[cid * CPJ:(cid + 1) * CPJ].astype(ml_dtypes.bfloat16)
            .reshape(1, CPJ), (N, CPJ))).view(np.uint8)
        in_maps.append({"ring_a": rings[0].view(np.int8),
                        "ring_b": rings[1].view(np.int8)})
    return in_maps


def _run(inputs, trace=False):
    if "nc" not in _CACHE:
        nc = _build_bass()
        nc.finalize()          # run bacc passes (reg alloc, wait splitting)
        _CACHE["nc"] = nc
    nc = _CACHE["nc"]
    in_maps = _prepare(inputs)
    res = run_bass_kernel_spmd(nc, in_maps, list(range(NCORES)), trace=trace)
    out = np.concatenate([res.results[c]["out"].astype(np.float32)
                          for c in range(NCORES)], axis=1)
    out = out.reshape(1, B, J * M)
    return out, res


def kernel(**inputs) -> np.ndarray:
    out, _ = _run(inputs, trace=False)
    return out


# revision 8
# speedup vs baseline: 1.2567x; 1.2567x over previous
"""BQQ linear inference kernel for 8 Trainium2 NeuronCores.

Math: after activation quantization, the whole BQQ op is linear in the
quantized input, so all four correction terms fold into one weight matrix:

    out[b, (j,m)] = X[b, (k,n)] @ W'[(k,n), (j,m)] + bias

where X = clip(round(x / act_scale), -127, 127) * act_scale and W' is a pure
function of the weights (Y_sign/Z_sign/scales/A), folded on the host.  The
device kernel per core is a pure streaming GEMM, tensor-parallel over the j
(output block) dim: 4 of 32 j-blocks per core.

Schedule notes (from trace analysis): the kernel is tensor-engine bound
(128 matmuls x ~216 ns).  HWDGE descriptor generation runs at ~1 descriptor
per 19 ns per ring (one descriptor per partition-row per trigger), so the
input stream uses FEW large triggers with long per-partition runs: x^T (int8)
and W (bf16, raw bytes) are interleaved per-k in one int8-typed DRAM tensor
per ring, chunks of 2-4 k-slabs, rings alternating k-pairs so slabs land in
k-order.  k0/k1 ship x as bf16 so the first matmuls need no cast; later x
slabs are upcast int8->bf16 on the DVE off the critical path (ints <= 127
are exact in bf16).  W regions are read through bf16 bitcast views.  A short
warmup matmul chain holds the PE clock (p-state) up through the DMA wait
without delaying the first real matmul.
"""

import numpy as np
import ml_dtypes

import concourse.bass as bass
import concourse.bacc as bacc
import concourse.mybir as mybir
from concourse.tile import TileContext
from concourse.tile_rust import add_dep_helper
from concourse.bass_utils import run_bass_kernel_spmd

F32 = mybir.dt.float32
BF16 = mybir.dt.bfloat16
I8 = mybir.dt.int8

P_, J, K, M, L, N = 2, 32, 32, 128, 16, 128
B = 512                  # tokens
NCORES = 8
JLOC = J // NCORES       # 4 j-blocks per core
CPJ = JLOC * M           # 512 output cols per core
QMAX = 127.0
WARMUP = 47
KSPLIT = 24              # k < KSPLIT: banks interleaved; then bank-by-bank

# The SDMA/HBM path drains chunks roughly in global trigger order, so the
# rings alternate k-pair chunks in k order (each chunk = one trigger = 128
# descriptors); later chunks escalate to quads once the GEMM has lookahead.
CHUNKS_A = [[0, 1], [4, 5], [8, 9], [12, 13], [16, 17, 18, 19],
            [24, 25, 26, 27]]
CHUNKS_B = [[2, 3], [6, 7], [10, 11], [14, 15], [20, 21, 22, 23],
            [28, 29, 30, 31]]
XBF = (0, 1)             # k-slabs whose x part ships as bf16 (2048 B slabs)


def _slab_bytes(k):
    return 2048 if k in XBF else 1536   # [x | w] bytes per partition


def _layout():
    """k -> (ring, byte_base) plus per-ring total bytes."""
    where = {}
    tot = [0, 0]
    for r, chunks in enumerate((CHUNKS_A, CHUNKS_B)):
        for ch in chunks:
            for k in ch:
                where[k] = (r, tot[r])
                tot[r] += _slab_bytes(k)
    return where, tot


WHERE, RING_BYTES = _layout()
BIAS_BASE = RING_BYTES[1]
RING_BYTES = [RING_BYTES[0], RING_BYTES[1] + 2 * CPJ]   # bias on ring B

_CACHE = {}


def _build_bass():
    nc = bacc.Bacc()
    a_d = nc.declare_dram_parameter("ring_a", [128, RING_BYTES[0]], I8,
                                    isOutput=False)
    b_d = nc.declare_dram_parameter("ring_b", [128, RING_BYTES[1]], I8,
                                    isOutput=False)
    out_d = nc.declare_dram_parameter("out", [B, CPJ], BF16, isOutput=True)

    with TileContext(nc) as tc:
        with tc.tile_pool(name="big", bufs=1) as big, \
             tc.tile_pool(name="sm", bufs=1) as sm, \
             tc.tile_pool(name="ot", bufs=4) as ot, \
             tc.tile_pool(name="psum", bufs=1, space="PSUM") as pp:
            pa = big.tile([128, RING_BYTES[0]], I8)    # ring A slabs
            pb = big.tile([128, RING_BYTES[1]], I8)    # ring B slabs + bias
            xbt = big.tile([128, K * B], BF16)         # upcast x^T (k >= 2)
            wz = sm.tile([128, 192], BF16)             # zeros for warmup
            wzms = nc.gpsimd.memset(wz[:], 0.0)

            psums = [pp.tile([128, CPJ], F32, name=f"psum{i}", tag=f"psum{i}")
                     for i in range(4)]
            wps = pp.tile([128, 64], F32, name="wps", tag="wps")

            def ring(k):
                r, base = WHERE[k]
                return (pa if r == 0 else pb), base

            # Phase A: a few big escalating triggers per ring, k-ascending.
            for tiles, dram, chunks in ((pa, a_d, CHUNKS_A), (pb, b_d,
                                                             CHUNKS_B)):
                eng = nc.sync if dram is a_d else nc.scalar
                for ch in chunks:
                    lo = WHERE[ch[0]][1]
                    hi = WHERE[ch[-1]][1] + _slab_bytes(ch[-1])
                    eng.dma_start(out=tiles[:, lo:hi], in_=dram[:, lo:hi])
            # bias (needed only at the tail) rides ring B last
            nc.scalar.dma_start(out=pb[:, BIAS_BASE:],
                                in_=b_d[:, BIAS_BASE:])
            bias_bc = pb[:, BIAS_BASE:].bitcast(BF16)

            # warmup matmuls paced off an early memset keep the PE p-state
            # ramped through the DMA wait
            for _ in range(WARMUP):
                mm = nc.tensor.matmul(
                    wps[:], lhsT=wz[:, 0:128],
                    rhs=wz[:, 128:192], start=True, stop=True)
                add_dep_helper(mm.ins, wzms.ins,
                               reason="pace PE warmup after memset")

            # x upcasts for k >= 2, in k order (k0/k1 are bf16 already)
            for k in range(2, K):
                t, base = ring(k)
                nc.vector.tensor_copy(
                    out=xbt[:, k * B:(k + 1) * B],
                    in_=t[:, base:base + 512])

            def lhsT(k, bb):
                if k in XBF:
                    t, base = ring(k)
                    return t[:, base + bb * 256:base + (bb + 1) * 256
                             ].bitcast(BF16)
                return xbt[:, k * B + bb * 128:k * B + (bb + 1) * 128]

            def rhs(k):
                t, base = ring(k)
                wb = base + (1024 if k in XBF else 512)
                return t[:, wb:wb + 1024].bitcast(BF16)

            # Phase B: the GEMM k-loop.  The last K - KSPLIT steps run
            # bank-by-bank so bank bb's epilogue (bias add + bf16 cast)
            # starts while bank bb+1 is still accumulating.
            def mm_step(k, bb, stop):
                nc.tensor.matmul(psums[bb][:], lhsT=lhsT(k, bb), rhs=rhs(k),
                                 start=(k == 0), stop=stop)

            def epilogue(bb):
                o = ot.tile([128, CPJ], BF16)
                nc.vector.tensor_add(o[:], psums[bb][:], bias_bc)
                rows = slice(bb * 128, (bb + 1) * 128)
                nc.sync.dma_start(out=out_d[rows, 0:256], in_=o[:, 0:256])
                nc.scalar.dma_start(out=out_d[rows, 256:512],
                                    in_=o[:, 256:512])

            for k in range(KSPLIT):
                for bb in range(4):
                    mm_step(k, bb, stop=False)
            for bb in range(4):
                for k in range(KSPLIT, K):
                    mm_step(k, bb, stop=(k == K - 1))
                epilogue(bb)
    return nc


def _fold_weights(Y_sign, Z_sign, Y_scale, Z_scale, A):
    """W[j,k,n,m]: everything linear in X folded into one matrix (fp32)."""
    ysc = Y_scale[..., 0, 0].astype(np.float32)      # (p,j,k)
    zsc = Z_scale[..., 0, 0].astype(np.float32)
    a0, a1, a2, a3 = (A[..., i].astype(np.float32) for i in range(4))
    Zs = Z_sign.astype(np.float32)
    Ys = Y_sign.astype(np.float32)
    # out1: sum_{p,l} a0*ysc*zsc * Z[l,n] * Y[m,l]  -> (j,k,n,m)
    t1 = np.einsum('pjkln,pjkml->pjknm', Zs, Ys, optimize=True)
    W = np.einsum('pjk,pjknm->jknm', a0 * ysc * zsc, t1, optimize=True)
    # out2: B_coef[j,k,m] broadcast over n
    Ysum = Ys.sum(-1) * ysc[..., None]               # (p,j,k,m)
    W += np.einsum('pjk,pjkm->jkm', a1, Ysum)[:, :, None, :]
    # out3: sum_p a2*zsc*Zsum[n] broadcast over m
    Zsum = Zs.sum(-2) * zsc[..., None]               # (p,j,k,n)
    W += np.einsum('pjk,pjkn->jkn', a2, Zsum)[:, :, :, None]
    # out4: D_coef[j,k] broadcast over n,m
    W += a3.sum(0)[:, :, None, None]
    return W


def _prepare(inputs):
    x = np.asarray(inputs["input"], dtype=np.float32)
    W = _fold_weights(np.asarray(inputs["Y_sign"], np.float32),
                      np.asarray(inputs["Z_sign"], np.float32),
                      np.asarray(inputs["Y_scale"], np.float32),
                      np.asarray(inputs["Z_scale"], np.float32),
                      np.asarray(inputs["A"], np.float32))
    bias = np.asarray(inputs["bias"], np.float32)

    # activation quantization on host (exact global max/min, RNE round)
    act_scale = max((float(x.max()) - float(x.min())) / (2.0 * QMAX), 1e-8)
    xq = np.clip(np.round(x / act_scale), -QMAX, QMAX)
    W = W * act_scale    # fold act_scale into the weights

    xtT = xq.reshape(B, K, N).transpose(2, 1, 0)     # [n, k, b] fp32
    x8 = xtT.astype(np.int8).view(np.uint8)          # int8 bytes
    xh = np.ascontiguousarray(xtT.astype(ml_dtypes.bfloat16)).view(np.uint8)

    in_maps = []
    for cid in range(NCORES):
        Wc = W[cid * JLOC:(cid + 1) * JLOC]          # [jl,k,n,m]
        wgt = np.ascontiguousarray(
            Wc.transpose(2, 1, 0, 3).reshape(N, K, CPJ).astype(
                ml_dtypes.bfloat16)).view(np.uint8)  # [n, k, 1024 bytes]
        rings = [np.empty((N, RING_BYTES[r]), np.uint8) for r in range(2)]
        for k in range(K):
            r, base = WHERE[k]
            if k in XBF:
                rings[r][:, base:base + 1024] = xh[:, k]
                rings[r][:, base + 1024:base + 2048] = wgt[:, k]
            else:
                rings[r][:, base:base + 512] = x8[:, k]
                rings[r][:, base + 512:base + 1536] = wgt[:, k]
        rings[1][:, BIAS_BASE:] = np.ascontiguousarray(np.broadcast_to(
            bias[cid * CPJ:(cid + 1) * CPJ].astype(ml_dtypes.bfloat16)
            .reshape(1, CPJ), (N, CPJ))).view(np.uint8)
        in_maps.append({"ring_a": rings[0].view(np.int8),
                        "ring_b": rings[1].view(np.int8)})
    return in_maps


def _run(inputs, trace=False):
    if "nc" not in _CACHE:
        nc = _build_bass()
        nc.finalize()          # run bacc passes (reg alloc, wait splitting)
        _CACHE["nc"] = nc
    nc = _CACHE["nc"]
    in_maps = _prepare(inputs)
    res = run_bass_kernel_spmd(nc, in_maps, list(range(NCORES)), trace=trace)
    out = np.concatenate([res.results[c]["out"].astype(np.float32)
                          for c in range(NCORES)], axis=1)
    out = out.reshape(1, B, J * M)
    return out, res


def kernel(**inputs) -> np.ndarray:
    out, _ = _run(inputs, trace=False)
    return out


# revision 11
# speedup vs baseline: 1.2941x; 1.0298x over previous
"""BQQ linear inference kernel for 8 Trainium2 NeuronCores.

Math: after activation quantization, the whole BQQ op is linear in the
quantized input, so all four correction terms fold into one weight matrix:

    out[b, (j,m)] = X[b, (k,n)] @ W'[(k,n), (j,m)] + bias

where X = clip(round(x / act_scale), -127, 127) * act_scale and W' is a pure
function of the weights (Y_sign/Z_sign/scales/A), folded on the host.  The
device kernel per core is a pure streaming GEMM, tensor-parallel over the j
(output block) dim: 4 of 32 j-blocks per core.

Schedule notes (from trace analysis): the kernel is tensor-engine bound
(128 matmuls x ~216 ns).  HWDGE descriptor generation runs at ~1 descriptor
per 19 ns per ring (one descriptor per partition-row per trigger), so the
input stream uses FEW large triggers with long per-partition runs: x^T (int8)
and W (bf16, raw bytes) are interleaved per-k in one int8-typed DRAM tensor
per ring, chunks of 2-4 k-slabs, rings alternating k-pairs so slabs land in
k-order.  k0/k1 ship x as bf16 so the first matmuls need no cast; later x
slabs are upcast int8->bf16 on the DVE off the critical path (ints <= 127
are exact in bf16).  W regions are read through bf16 bitcast views.  A short
warmup matmul chain holds the PE clock (p-state) up through the DMA wait
without delaying the first real matmul.
"""

import numpy as np
import ml_dtypes

import concourse.bass as bass
import concourse.bacc as bacc
import concourse.mybir as mybir
from concourse.tile import TileContext
from concourse.tile_rust import add_dep_helper
from concourse.bass_utils import run_bass_kernel_spmd

F32 = mybir.dt.float32
BF16 = mybir.dt.bfloat16
I8 = mybir.dt.int8

P_, J, K, M, L, N = 2, 32, 32, 128, 16, 128
B = 512                  # tokens
NCORES = 8
JLOC = J // NCORES       # 4 j-blocks per core
CPJ = JLOC * M           # 512 output cols per core
QMAX = 127.0
WARMUP = 38
KSPLIT = 24              # k < KSPLIT: banks interleaved; then bank-by-bank

# The SDMA/HBM path drains chunks roughly in global trigger order and a
# chunk completes only when the slowest of its 16 queues finishes, so the
# head chunks are tiny (single k), escalating to pairs and quads once the
# GEMM has lookahead.  Rings alternate chunks in k order.
CHUNKS_A = [[0], [2, 3], [6, 7], [10, 11, 12, 13], [18, 19, 20, 21],
            [26, 27, 28, 29]]
CHUNKS_B = [[1], [4, 5], [8, 9], [14, 15, 16, 17], [22, 23, 24, 25],
            [30, 31]]
XBF = (0, 1)             # k-slabs whose x part ships as bf16 (2048 B slabs)


def _slab_bytes(k):
    return 2048 if k in XBF else 1536   # [x | w] bytes per partition


def _layout():
    """k -> (ring, byte_base) plus per-ring total bytes."""
    where = {}
    tot = [0, 0]
    for r, chunks in enumerate((CHUNKS_A, CHUNKS_B)):
        for ch in chunks:
            for k in ch:
                where[k] = (r, tot[r])
                tot[r] += _slab_bytes(k)
    return where, tot


WHERE, RING_BYTES = _layout()
BIAS_BASE = RING_BYTES[1]
RING_BYTES = [RING_BYTES[0], RING_BYTES[1] + 2 * CPJ]   # bias on ring B

_CACHE = {}


def _build_bass():
    nc = bacc.Bacc()
    a_d = nc.declare_dram_parameter("ring_a", [128, RING_BYTES[0]], I8,
                                    isOutput=False)
    b_d = nc.declare_dram_parameter("ring_b", [128, RING_BYTES[1]], I8,
                                    isOutput=False)
    out_d = nc.declare_dram_parameter("out", [B, CPJ], BF16, isOutput=True)

    with TileContext(nc) as tc:
        with tc.tile_pool(name="big", bufs=1) as big, \
             tc.tile_pool(name="sm", bufs=1) as sm, \
             tc.tile_pool(name="ot", bufs=4) as ot, \
             tc.tile_pool(name="psum", bufs=1, space="PSUM") as pp:
            pa = big.tile([128, RING_BYTES[0]], I8)    # ring A slabs
            pb = big.tile([128, RING_BYTES[1]], I8)    # ring B slabs + bias
            xbt = big.tile([128, K * B], BF16)         # upcast x^T (k >= 2)
            wz = sm.tile([128, 192], BF16)             # zeros for warmup
            wzms = nc.gpsimd.memset(wz[:], 0.0)

            psums = [pp.tile([128, CPJ], F32, name=f"psum{i}", tag=f"psum{i}")
                     for i in range(4)]
            wps = pp.tile([128, 64], F32, name="wps", tag="wps")

            def ring(k):
                r, base = WHERE[k]
                return (pa if r == 0 else pb), base

            # Phase A: a few big escalating triggers per ring, k-ascending.
            for tiles, dram, chunks in ((pa, a_d, CHUNKS_A), (pb, b_d,
                                                             CHUNKS_B)):
                eng = nc.sync if dram is a_d else nc.scalar
                for ch in chunks:
                    lo = WHERE[ch[0]][1]
                    hi = WHERE[ch[-1]][1] + _slab_bytes(ch[-1])
                    if dram is b_d and ch is chunks[-1]:
                        hi = RING_BYTES[1]   # bias rides the last trigger
                    eng.dma_start(out=tiles[:, lo:hi], in_=dram[:, lo:hi])
            bias_bc = pb[:, BIAS_BASE:].bitcast(BF16)

            # warmup matmuls paced off an early memset keep the PE p-state
            # ramped through the DMA wait
            for _ in range(WARMUP):
                mm = nc.tensor.matmul(
                    wps[:], lhsT=wz[:, 0:128],
                    rhs=wz[:, 128:192], start=True, stop=True)
                add_dep_helper(mm.ins, wzms.ins,
                               reason="pace PE warmup after memset")

            # x upcasts for k >= 2, in k order (k0/k1 are bf16 already)
            for k in range(2, K):
                t, base = ring(k)
                nc.vector.tensor_copy(
                    out=xbt[:, k * B:(k + 1) * B],
                    in_=t[:, base:base + 512])

            def lhsT(k, bb):
                if k in XBF:
                    t, base = ring(k)
                    return t[:, base + bb * 256:base + (bb + 1) * 256
                             ].bitcast(BF16)
                return xbt[:, k * B + bb * 128:k * B + (bb + 1) * 128]

            def rhs(k):
                t, base = ring(k)
                wb = base + (1024 if k in XBF else 512)
                return t[:, wb:wb + 1024].bitcast(BF16)

            # Phase B: the GEMM k-loop.  The last K - KSPLIT steps run
            # bank-by-bank so bank bb's epilogue (bias add + bf16 cast)
            # starts while bank bb+1 is still accumulating.
            def mm_step(k, bb, stop):
                nc.tensor.matmul(psums[bb][:], lhsT=lhsT(k, bb), rhs=rhs(k),
                                 start=(k == 0), stop=stop)

            def epilogue(bb):
                o = ot.tile([128, CPJ], BF16)
                nc.vector.tensor_add(o[:], psums[bb][:], bias_bc)
                rows = slice(bb * 128, (bb + 1) * 128)
                nc.sync.dma_start(out=out_d[rows, 0:256], in_=o[:, 0:256])
                nc.scalar.dma_start(out=out_d[rows, 256:512],
                                    in_=o[:, 256:512])

            for k in range(KSPLIT):
                for bb in range(4):
                    mm_step(k, bb, stop=False)
            for bb in range(4):
                for k in range(KSPLIT, K):
                    mm_step(k, bb, stop=(k == K - 1))
                epilogue(bb)
    return nc


def _fold_weights(Y_sign, Z_sign, Y_scale, Z_scale, A):
    """W[j,k,n,m]: everything linear in X folded into one matrix (fp32)."""
    ysc = Y_scale[..., 0, 0].astype(np.float32)      # (p,j,k)
    zsc = Z_scale[..., 0, 0].astype(np.float32)
    a0, a1, a2, a3 = (A[..., i].astype(np.float32) for i in range(4))
    Zs = Z_sign.astype(np.float32)
    Ys = Y_sign.astype(np.float32)
    # out1: sum_{p,l} a0*ysc*zsc * Z[l,n] * Y[m,l]  -> (j,k,n,m)
    t1 = np.einsum('pjkln,pjkml->pjknm', Zs, Ys, optimize=True)
    W = np.einsum('pjk,pjknm->jknm', a0 * ysc * zsc, t1, optimize=True)
    # out2: B_coef[j,k,m] broadcast over n
    Ysum = Ys.sum(-1) * ysc[..., None]               # (p,j,k,m)
    W += np.einsum('pjk,pjkm->jkm', a1, Ysum)[:, :, None, :]
    # out3: sum_p a2*zsc*Zsum[n] broadcast over m
    Zsum = Zs.sum(-2) * zsc[..., None]               # (p,j,k,n)
    W += np.einsum('pjk,pjkn->jkn', a2, Zsum)[:, :, :, None]
    # out4: D_coef[j,k] broadcast over n,m
    W += a3.sum(0)[:, :, None, None]
    return W


def _prepare(inputs):
    x = np.asarray(inputs["input"], dtype=np.float32)
    W = _fold_weights(np.asarray(inputs["Y_sign"], np.float32),
                      np.asarray(inputs["Z_sign"], np.float32),
                      np.asarray(inputs["Y_scale"], np.float32),
                      np.asarray(inputs["Z_scale"], np.float32),
                      np.asarray(inputs["A"], np.float32))
    bias = np.asarray(inputs["bias"], np.float32)

    # activation quantization on host (exact global max/min, RNE round)
    act_scale = max((float(x.max()) - float(x.min())) / (2.0 * QMAX), 1e-8)
    xq = np.clip(np.round(x / act_scale), -QMAX, QMAX)
    W = W * act_scale    # fold act_scale into the weights

    xtT = xq.reshape(B, K, N).transpose(2, 1, 0)     # [n, k, b] fp32
    x8 = xtT.astype(np.int8).view(np.uint8)          # int8 bytes
    xh = np.ascontiguousarray(xtT.astype(ml_dtypes.bfloat16)).view(np.uint8)

    in_maps = []
    for cid in range(NCORES):
        Wc = W[cid * JLOC:(cid + 1) * JLOC]          # [jl,k,n,m]
        wgt = np.ascontiguousarray(
            Wc.transpose(2, 1, 0, 3).reshape(N, K, CPJ).astype(
                ml_dtypes.bfloat16)).view(np.uint8)  # [n, k, 1024 bytes]
        rings = [np.empty((N, RING_BYTES[r]), np.uint8) for r in range(2)]
        for k in range(K):
            r, base = WHERE[k]
            if k in XBF:
                rings[r][:, base:base + 1024] = xh[:, k]
                rings[r][:, base + 1024:base + 2048] = wgt[:, k]
            else:
                rings[r][:, base:base + 512] = x8[:, k]
                rings[r][:, base + 512:base + 1536] = wgt[:, k]
        rings[1][:, BIAS_BASE:] = np.ascontiguousarray(np.broadcast_to(
            bias[cid * CPJ:(cid + 1) * CPJ].astype(ml_dtypes.bfloat16)
            .reshape(1, CPJ), (N, CPJ))).view(np.uint8)
        in_maps.append({"ring_a": rings[0].view(np.int8),
                        "ring_b": rings[1].view(np.int8)})
    return in_maps


def _run(inputs, trace=False):
    if "nc" not in _CACHE:
        nc = _build_bass()
        nc.finalize()          # run bacc passes (reg alloc, wait splitting)
        _CACHE["nc"] = nc
    nc = _CACHE["nc"]
    in_maps = _prepare(inputs)
    res = run_bass_kernel_spmd(nc, in_maps, list(range(NCORES)), trace=trace)
    out = np.concatenate([res.results[c]["out"].astype(np.float32)
                          for c in range(NCORES)], axis=1)
    out = out.reshape(1, B, J * M)
    return out, res


def kernel(**inputs) -> np.ndarray:
    out, _ = _run(inputs, trace=False)
    return out
